# revision 79
# baseline (speedup 1.0000x reference)
"""BitNet attention layer on 8 Trainium2 NeuronCores.

Tensor-parallel over heads: core i owns heads {2i, 2i+1}. Each core:
  - computes q^T,k^T (feature-major) + v (natural) for its heads via
    fp8e4 DoubleRow matmuls: ternary W^T (fp8-exact, duplicated pair) against
    hi/lo fp8 residual split of x^T -> full fp32-class precision at 0.5
    cycles/row (2x fp32r)
  - RoPE on q^T/k^T in bf16 (partition-dim rotate-half, sign folded into sin)
  - causal attention with transposed scores S^T[k,q]; probs in bf16; softmax
    denominator via bf16 accumulate (DVE) + partition_all_reduce (Pool);
    diagonal blocks masked with a 128-wide tril, and the fully-masked
    [0,off) column range of diagonal chunks is skipped in ctx/denominator
  - ctx split into fp8 hi/lo residual -> o_proj as DoubleRow against
    duplicated ternary w_o^T; output scale conversions rotate across engines
Host sums the 8 fp16 partials.
"""
import os
import sys

import numpy as np

try:
    import concourse.bass as bass
except ImportError:
    sys.path.insert(0, "/opt/trn_rl_repo")
    import concourse.bass as bass

import concourse.mybir as mybir
import concourse.tile as tile
from concourse import bacc, bass_isa
from concourse.bass_utils import run_bass_kernel_spmd

F32 = mybir.dt.float32
F32R = mybir.dt.float32r
F16 = mybir.dt.float16
BF16 = mybir.dt.bfloat16
F8 = mybir.dt.float8e4
DR = mybir.MatmulPerfMode.DoubleRow

S = 2048          # sequence length
H = 2048          # hidden
D = 128           # head dim
NCORES = 8
HPC = 2           # heads per core
OC = 3 * HPC * D  # 768 per-core projection output features (q|k|v)
ST = 512          # seq tile for projection rhs / attention qi tile
NST = S // ST     # 4
HC = H // 128     # 16 h-chunks
HG = 2            # h-chunk group size (DMA granularity)
NG = HC // HG     # 4 groups
NKJ = S // 128    # 16 kj chunks
ROPE_BASE = 10000.0

_built = None
_PHASES = os.environ.get("KPH", "ABC")


def _build(timing=False):
    nc = bacc.Bacc("TRN2", target_bir_lowering=False, debug=False,
                   dynamic_dma_scratch_size=4096)

    if timing:
        # timing variant: identical device work, but big tensors live in
        # internal DRAM (garbage data) so per-call host<->device transfer is
        # tiny and wall-clock deltas measure the NEFF itself.
        xt_d = nc.dram_tensor("xt8_i", [2 * H, S], F8)
        wt_d = nc.dram_tensor("wt8_i", [2 * H, OC], F8)
        wot_d = nc.dram_tensor("wot8_i", [2 * HPC * D, H], F8)
        cos_d = nc.dram_tensor("cost_i", [D, S], BF16)
        sin_d = nc.dram_tensor("sins_i", [D, S], BF16)
        tri_d = nc.dram_tensor("tri_i", [128, 128], BF16)
        out_d = nc.dram_tensor("out_i", [S, H], F16)
        out_x = nc.declare_dram_parameter("out", [128, H], F16, isOutput=True)
    else:
        xt_d = nc.declare_dram_parameter("xt8", [2 * H, S], F8, isOutput=False)
        wt_d = nc.declare_dram_parameter("wt8", [2 * H, OC], F8, isOutput=False)
        wot_d = nc.declare_dram_parameter("wot8", [2 * HPC * D, H], F8,
                                          isOutput=False)
        cos_d = nc.declare_dram_parameter("cost", [D, S], BF16, isOutput=False)
        sin_d = nc.declare_dram_parameter("sins", [D, S], BF16, isOutput=False)
        tri_d = nc.declare_dram_parameter("tri", [128, 128], BF16,
                                          isOutput=False)
        out_d = nc.declare_dram_parameter("out", [S, H], F16, isOutput=True)

    _dbg = os.environ.get("KDBG") and not timing
    if _dbg:
        dbg_qk_d = nc.declare_dram_parameter("dbg_qk", [D, ST], BF16,
                                             isOutput=True)
        dbg_v_d = nc.declare_dram_parameter("dbg_v", [128, 4 * HPC * D], BF16,
                                            isOutput=True)
        dbg_pt_d = nc.declare_dram_parameter("dbg_pt", [128, 4 * ST], BF16,
                                             isOutput=True)
        dbg_bp_d = nc.declare_dram_parameter("dbg_bp", [128, ST], F32,
                                             isOutput=True)
        dbg_c8_d = nc.declare_dram_parameter("dbg_c8", [D, 2 * ST], F8,
                                             isOutput=True)
    osq_d = nc.declare_dram_parameter("osq", [128, 128], F32R, isOutput=False)
    # exp scale (s_p^2/sqrt(D)) and output scale (s_p*s_o) are runtime values;
    # pass them as tiny per-partition inputs instead of baking into the NEFF.
    esc_d = nc.declare_dram_parameter("esc", [128, 1], F32, isOutput=False)
    osc_d = nc.declare_dram_parameter("osc", [128, 1], F32, isOutput=False)

    with tile.TileContext(nc) as tc, nc.allow_low_precision(
        reason="fp8 DoubleRow residual matmuls + bf16 elementwise"
    ):
        with tc.tile_pool(name="const", bufs=1) as cpool, \
             tc.tile_pool(name="qkv", bufs=1) as qpool, \
             tc.tile_pool(name="ctx", bufs=1) as xpool, \
             tc.tile_pool(name="wo", bufs=1) as wopool, \
             tc.tile_pool(name="ob", bufs=7) as opool:
            cosb = cpool.tile([D, S], BF16)
            sinb = cpool.tile([D, S], BF16)
            tri = cpool.tile([128, 128], BF16)
            esc = cpool.tile([128, 1], F32)
            osc = cpool.tile([128, 1], F32)
            osq = cpool.tile([128, 128], F32R)
            wot = wopool.tile([128, 2 * HPC, H], F8)

            # persistent per-head tensors, tiled per seq-tile for fine deps
            qk = [[qpool.tile([D, ST], BF16, name=f"qk{oc}_{st}")
                   for st in range(NST)] for oc in range(4)]
            v_sb = [qpool.tile([128, ST // 128, HPC * D], BF16, name=f"v{st}")
                    for st in range(NST)]
            ctx8 = [[xpool.tile([D, 2, ST], F8, name=f"ctx{h}_{t}")
                     for t in range(NST)] for h in range(HPC)]

            # ---------------- Phase A: qkv projection + RoPE ----------------
            if "A" in _PHASES:
             with tc.tile_pool(name="wt", bufs=1) as wpool, \
                 tc.tile_pool(name="xt", bufs=2) as xtpool, \
                 tc.tile_pool(name="ropet", bufs=3) as rpool, \
                 tc.tile_pool(name="psA", bufs=1, space="PSUM") as psA, \
                 tc.tile_pool(name="psV", bufs=1, space="PSUM") as psV:
                wt = [wpool.tile([128, HG * 2, OC], F8, name=f"wt{g}")
                      for g in range(NG)]

                def xt_fetch(st):
                    ssl = slice(st * ST, (st + 1) * ST)
                    xt = [xtpool.tile([128, HG * 2, ST], F8, name=f"xt{g}")
                          for g in range(NG)]
                    for g in range(NG):
                        if st == 0:
                            # interleave weight and first-x DMAs so group-0
                            # matmuls can start after ~2 transfers
                            nc.sync.dma_start(
                                wt[g][:],
                                wt_d[g * HG * 256:(g + 1) * HG * 256].rearrange(
                                    "(hc hp) o -> hp hc o", hp=128))
                        nc.sync.dma_start(
                            xt[g][:],
                            xt_d[g * HG * 256:(g + 1) * HG * 256,
                                 ssl].rearrange(
                                "(hc hp) s -> hp hc s", hp=128))
                    return xt

                xt_cur = xt_fetch(0)
                xt_next = xt_fetch(1)

                for st in range(NST):
                    ssl = slice(st * ST, (st + 1) * ST)
                    xt = xt_cur
                    if st + 2 < NST:
                        xt_cur = xt_next
                        xt_next = xt_fetch(st + 2)
                    else:
                        xt_cur = xt_next
                    if st == 0:
                        # tables/consts behind the xt(st2) prefetch: RoPE and
                        # phase B consume them well after phase A's stream,
                        # but they must be emitted before their readers
                        nc.sync.dma_start(cosb[:], cos_d[:])
                        nc.sync.dma_start(sinb[:], sin_d[:])
                        nc.sync.dma_start(tri[:], tri_d[:])
                        nc.sync.dma_start(esc[:], esc_d[:])
                        nc.sync.dma_start(osq[:], osq_d[:])
                        nc.sync.dma_start(osc[:], osc_d[:])
                    if st == 1:
                        # o_proj weights are not needed until C(t0), well
                        # after phase A; keep them out of the startup stream
                        nc.sync.dma_start(
                            wot[:],
                            wot_d.rearrange("(ct cp) o -> cp ct o", cp=128))

                    # group-major accumulation: 4 q/k psums + v psum alive,
                    # each fed by all 16 h-chunks in (g, c) order
                    ps = [psA.tile([128, ST], F32, name=f"ps{oc}")
                          for oc in range(4)]
                    psv = psV.tile([128, ST // 128, HPC * D], F32, name="psv")
                    for g in range(NG):
                        for oc in range(4):
                            for c in range(HG):
                                nc.tensor.matmul(
                                    ps[oc][:],
                                    wt[g][:, 2 * c:2 * c + 2,
                                          oc * 128:(oc + 1) * 128],
                                    xt[g][:, 2 * c:2 * c + 2, :],
                                    start=(g == 0 and c == 0),
                                    stop=(g == NG - 1 and c == HG - 1),
                                    perf_mode=DR)
                        for sc in range(ST // 128):
                            for c in range(HG):
                                # psum accumulation groups are per 2KB bank:
                                # sc pairs (0,1) and (2,3) share a bank, so
                                # start/stop only on the bank's first/last write
                                nc.tensor.matmul(
                                    psv[:, sc, :],
                                    xt[g][:, 2 * c:2 * c + 2,
                                          sc * 128:(sc + 1) * 128],
                                    wt[g][:, 2 * c:2 * c + 2, 4 * 128:],
                                    start=(g == 0 and c == 0 and sc % 2 == 0),
                                    stop=(g == NG - 1 and c == HG - 1
                                          and sc % 2 == 1),
                                    perf_mode=DR)

                    # RoPE into qk[oc][st]. Walrus rules: only copies may
                    # convert f32->bf16 on DVE; crossed-partition tensor ops
                    # need all-same dtype; Pool cannot touch f32. So: DVE
                    # copy psum->bf16, DVE crossed bf16 half-copies for the
                    # rotate, DVE bf16 sin-mul (2x), and the cos-mul + add on
                    # the otherwise-idle Pool engine.
                    for oc in range(4):
                        dst = qk[oc][st]
                        qs = rpool.tile([128, ST], BF16, name="qs")
                        qw = rpool.tile([128, ST], BF16, name="qw")
                        t2 = rpool.tile([128, ST], BF16, name="t2")
                        if oc % 2 == 0:
                            nc.vector.tensor_copy(qs[:], ps[oc][:])
                        else:
                            nc.scalar.copy(qs[:], ps[oc][:])
                        nc.vector.tensor_copy(qw[0:64, :], qs[64:128, :])
                        nc.vector.tensor_copy(qw[64:128, :], qs[0:64, :])
                        nc.vector.tensor_mul(t2[:], qw[:], sinb[:, ssl])
                        nc.gpsimd.tensor_mul(dst[:], qs[:], cosb[:, ssl])
                        nc.vector.tensor_add(dst[:], dst[:], t2[:])

                    for sc in range(ST // 128):
                        # independent copies: alternate engines
                        if sc % 2 == 0:
                            nc.vector.tensor_copy(v_sb[st][:, sc, :],
                                                  psv[:, sc, :])
                        else:
                            nc.scalar.copy(v_sb[st][:, sc, :], psv[:, sc, :])
                    if _dbg and st == 0:
                        nc.sync.dma_start(dbg_qk_d[:], qk[0][0][:])
                        nc.sync.dma_start(
                            dbg_v_d.rearrange("p (sc f) -> p sc f",
                                              f=HPC * D), v_sb[0][:])

            # ---------- Phase B+C: attention + o_proj, interleaved ----------
            if "B" in _PHASES:
             with tc.tile_pool(name="pt", bufs=3) as ptpool, \
                 tc.tile_pool(name="rden", bufs=8) as dpool, \
                 tc.tile_pool(name="psS", bufs=2, space="PSUM") as psS, \
                 tc.tile_pool(name="psC", bufs=1, space="PSUM") as psC, \
                 tc.tile_pool(name="psO", bufs=3, space="PSUM") as psO:
                _oconv = [0]
                for t in range(NST):
                    for h in range(HPC):
                        nkj = 4 * (t + 1)
                        pt = ptpool.tile([128, NKJ, ST], BF16)
                        den = dpool.tile([128, ST], BF16, name="den")
                        # scores in j pairs sharing a 2-bank psum tile; one
                        # batched exp per pair
                        for j0 in range(0, nkj, 2):
                            # columns [0, off_p) of this pair are fully
                            # causally masked -> skip them in scores + exp
                            off_p = max(0, 128 * j0 - ST * t)
                            sp = psS.tile([128, 2, ST], F32, name="sp")
                            for dj in range(2):
                                j = j0 + dj
                                nc.tensor.matmul(
                                    sp[:, dj, off_p:],
                                    qk[2 + h][j // 4][:, (j % 4) * 128:
                                                      (j % 4 + 1) * 128],
                                    qk[h][t][:, off_p:],
                                    start=True, stop=True)
                            # probs (unnormalized): exp(esc * scores)
                            nc.scalar.activation(
                                pt[:, j0:j0 + 2, off_p:], sp[:, :, off_p:],
                                mybir.ActivationFunctionType.Exp,
                                bias=0.0, scale=esc[:])
                            for dj in range(2):
                                j = j0 + dj
                                off = 128 * j - ST * t
                                if off >= 0:
                                    # partial boundary block: 128-wide tril;
                                    # columns [0,off) are fully masked and
                                    # simply skipped downstream
                                    nc.vector.tensor_mul(
                                        pt[:, j, off:off + 128],
                                        pt[:, j, off:off + 128],
                                        tri[:])
                                # denominator accumulate (bf16, 2x DVE)
                                if j == 0:
                                    nc.vector.tensor_copy(den[:], pt[:, 0, :])
                                elif off > 0:
                                    nc.vector.tensor_add(
                                        den[:, off:], den[:, off:],
                                        pt[:, j, off:])
                                else:
                                    nc.vector.tensor_add(den[:], den[:],
                                                         pt[:, j, :])
                        # ctx^T[d, qi] accumulate over kj (diag chunks only
                        # touch their live column range)
                        cp = psC.tile([128, ST], F32, name="cp")
                        for j in range(nkj):
                            off = max(0, 128 * j - ST * t)
                            nc.tensor.matmul(
                                cp[:, off:],
                                v_sb[j // 4][:, j % 4, h * D:(h + 1) * D],
                                pt[:, j, off:],
                                start=(j == 0), stop=(j == nkj - 1))
                        # denominator: bf16 partition reduce on Pool,
                        # then reciprocal + normalize + fp8 hi/lo split.
                        # Chained in column chunks so o_proj on the matching
                        # 128-row block can start early; finest chunks on the
                        # last tile where nothing else hides the latency.
                        bp = dpool.tile([128, ST], F32, name="bp")
                        rbp = dpool.tile([128, ST], F32, name="rbp")
                        t16f = dpool.tile([128, ST], F32, name="t16f")
                        t16 = dpool.tile([128, ST], BF16, name="t16")
                        QW = 128 if t == NST - 1 else ST
                        for q0 in range(0, ST, QW):
                            qs_ = slice(q0, q0 + QW)
                            nc.gpsimd.partition_all_reduce(
                                bp[:, qs_], den[:, qs_], channels=128,
                                reduce_op=bass_isa.ReduceOp.add)
                            nc.vector.reciprocal(rbp[:, qs_], bp[:, qs_])
                            nc.vector.tensor_mul(t16f[:, qs_], cp[:, qs_],
                                                 rbp[:, qs_])
                            nc.vector.tensor_copy(t16[:, qs_], t16f[:, qs_])
                            nc.vector.tensor_copy(ctx8[h][t][:, 0, qs_],
                                                  t16[:, qs_])
                            nc.vector.tensor_sub(ctx8[h][t][:, 1, qs_],
                                                 t16[:, qs_],
                                                 ctx8[h][t][:, 0, qs_])
                        if _dbg and t == 0 and h == 0:
                            nc.sync.dma_start(
                                dbg_pt_d.rearrange("p (j s) -> p j s", s=ST),
                                pt[:, 0:4, :])
                            nc.sync.dma_start(dbg_bp_d[:], bp[:])
                        if _dbg and t == 0 and h == 0:
                            nc.sync.dma_start(
                                dbg_c8_d.rearrange("p (two s) -> p two s",
                                                   s=ST),
                                ctx8[0][0][:])

                    # o_proj rows for this t (ctx8[*][t] complete)
                    if "C" in _PHASES:
                        for sc in range(4 * t, 4 * t + 4):
                            for half in range(2):
                                ob = opool.tile([128, H // 2], F16)
                                for oth in range(2):
                                    ot = half * 2 + oth
                                    po = psO.tile([128, ST], F32, name="po")
                                    for cc in range(HPC):
                                        nc.tensor.matmul(
                                            po[:],
                                            ctx8[cc][t][:, :, (sc % 4) * 128:
                                                        (sc % 4 + 1) * 128],
                                            wot[:, 2 * cc:2 * cc + 2,
                                                ot * ST:(ot + 1) * ST],
                                            start=(cc == 0),
                                            stop=(cc == HPC - 1),
                                            perf_mode=DR)
                                    # scale+f16 conversion: only DVE
                                    # tensor_scalar and Act may convert f32
                                    eng = _oconv[0] % 3
                                    _oconv[0] += 1
                                    obs = ob[:, oth * ST:(oth + 1) * ST]
                                    if eng == 0:
                                        nc.vector.tensor_scalar_mul(
                                            obs, po[:], osc[:])
                                    else:
                                        nc.scalar.activation(
                                            obs, po[:],
                                            mybir.ActivationFunctionType.Copy,
                                            bias=0.0, scale=osc[:])
                                nc.sync.dma_start(
                                    out_d[sc * 128:(sc + 1) * 128,
                                          half * (H // 2):(half + 1) * (H // 2)],
                                    ob[:])

            if timing:
                nc.sync.dma_start(out_x[:], out_d[S - 128:, :])

    nc.compile()
    return nc


def _host_prep(hidden_states, w_proj, w_o):
    import ml_dtypes
    E4 = ml_dtypes.float8_e4m3

    x = np.asarray(hidden_states, dtype=np.float32).reshape(S, H)
    w_proj = np.asarray(w_proj, dtype=np.float32)
    w_o = np.asarray(w_o, dtype=np.float32)

    # BitNet b1.58 per-tensor absmean quantization (ternary, scale factored out)
    s_p = np.float32(np.mean(np.abs(w_proj), dtype=np.float32)) + np.float32(1e-5)
    s_o = np.float32(np.mean(np.abs(w_o), dtype=np.float32)) + np.float32(1e-5)
    tp = np.clip(np.round(w_proj / s_p), -1.0, 1.0).astype(np.float32)
    to = np.clip(np.round(w_o / s_o), -1.0, 1.0).astype(np.float32)

    xt = np.ascontiguousarray(x.T)                      # [H, S]
    # hi/lo fp8 residual split: xt == hi + lo to ~0.2% per element
    xt_hi = xt.astype(E4)
    xt_lo = (xt - xt_hi.astype(np.float32)).astype(E4)
    # [(ho two hp), S]: per 128-row chunk, hi block then lo block
    xt8 = np.ascontiguousarray(
        np.stack([xt_hi.reshape(HC, 128, S), xt_lo.reshape(HC, 128, S)],
                 axis=1).reshape(2 * H, S))

    # RoPE tables, feature-major, rotate-half sign folded into sin
    inv_freq = (1.0 / (ROPE_BASE ** (np.arange(0, D, 2, dtype=np.float32) / D))
                ).astype(np.float32)
    t = np.arange(S, dtype=np.float32)
    freqs = np.outer(inv_freq, t).astype(np.float32)    # [64, S]
    cosT = np.concatenate([np.cos(freqs), np.cos(freqs)], 0)
    sinS = np.concatenate([-np.sin(freqs), np.sin(freqs)], 0)
    cosT = cosT.astype(ml_dtypes.bfloat16)
    sinS = sinS.astype(ml_dtypes.bfloat16)

    # 128-wide tril mask: tri[p, c] = 1 if p <= c
    p = np.arange(128)[:, None]
    xx = np.arange(128)[None, :]
    tri = (p <= xx).astype(ml_dtypes.bfloat16)

    esc = np.full((128, 1), s_p * s_p / np.sqrt(np.float32(D)), np.float32)
    osc = np.full((128, 1), s_p * s_o, np.float32)

    in_maps = []
    for c in range(NCORES):
        r = slice(c * HPC * D, (c + 1) * HPC * D)       # 256 features
        wt_c = np.ascontiguousarray(
            np.concatenate([tp[:H][r], tp[H:2 * H][r], tp[2 * H:][r]], 0).T
        ).astype(E4)                                     # [H, OC] fp8-exact
        wtr = wt_c.reshape(HC, 128, OC)
        wt8_c = np.ascontiguousarray(
            np.stack([wtr, wtr], axis=1).reshape(2 * H, OC))  # dup pair
        wot_c = np.ascontiguousarray(to[:, r].T).astype(E4)   # [256, H]
        wor = wot_c.reshape(HPC, 128, H)
        wot8_c = np.ascontiguousarray(
            np.stack([wor, wor], axis=1).reshape(2 * HPC * 128, H))
        in_maps.append({
            "xt8": xt8, "wt8": wt8_c, "wot8": wot8_c, "cost": cosT,
            "sins": sinS, "tri": tri, "esc": esc, "osc": osc,
            "osq": np.ones((128, 128), np.float32),
        })
    return in_maps


def kernel(hidden_states, attention_mask, w_proj, w_o):
    global _built
    if _built is None:
        _built = _build()
    nc = _built
    in_maps = _host_prep(hidden_states, w_proj, w_o)
    res = run_bass_kernel_spmd(nc, in_maps, core_ids=list(range(NCORES)))
    acc = np.zeros((S, H), np.float32)
    for c in range(NCORES):
        acc += res.results[c]["out"].astype(np.float32)
    return acc.reshape(1, S, H)
